# revision 1
# baseline (speedup 1.0000x reference)
"""Trainium2 Bass kernel for BatchedStarNetAttentionBlock.

Strategy: data-parallel over ordering segments (attention is block-diagonal,
never crosses segment boundaries). Each of the 8 cores gets a subset of
segments, padded to a shared static structure so one SPMD program serves all
cores. No collectives.

On-device layout: activations are kept transposed, xT[d, n] with the feature
dim on partitions (2 tiles of 128), so every linear layer is a natural
matmul (lhsT = weight chunk [k,j], rhs = xT chunk [k,n]). Scores are computed
directly in transposed form S.T = kT.T @ qT (keys on partitions), so softmax
exp output P.T feeds the PV matmul without any transpose. PV is col-tiled by
head (tile_position=(0,32h)) so attention output lands as oT[d, n] in PSUM.
Denominators come from ones-matmuls writing a row-replicated bank with the
same layout as oT, so normalization is a single fused multiply+copy.
"""

import sys

for _p in ("/opt/trn_rl_repo",):
    if _p not in sys.path:
        sys.path.insert(0, _p)

import numpy as np
import ml_dtypes

import bass_rust as _bass_rust

import concourse.bass as bass
import concourse.tile as tile
from concourse import bacc
from concourse import mybir
from concourse.bass_utils import run_bass_kernel_spmd
from concourse.hw_specs import get_activation_tables


class _Bacc(bacc.Bacc):
    """Bacc whose activation-table planner prefers the set that contains
    exp+ln+square+identity together, so per-layernorm Ln/Exp pairs do not
    ping-pong ACT table loads (~2.6us per switch)."""

    def insert_act_table_loads(self):
        has_activation = any(
            isinstance(i, mybir.InstActivation)
            for b in self.main_func.blocks
            for i in b.instructions
        )
        if not has_activation:
            return
        tables = list(get_activation_tables(self.m.arch).items())
        # The planner emits act_func_set_id = position in this list, so
        # positions must stay aligned with act_info.json. Narrow the match
        # sets instead: position 0 claims only tanh; other sets before
        # natural_log_exp_and_others claim nothing; so exp/ln/square/
        # identity/copy all resolve to the one set that has them all.
        pref = "natural_log_exp_and_others"
        TANH = mybir.ActivationFunctionType.Tanh
        doctored = []
        seen_pref = False
        for name, fns in tables:
            if name == pref:
                seen_pref = True
                doctored.append((name, fns))
            elif not seen_pref:
                doctored.append((name, {TANH} & fns))
            else:
                doctored.append((name, fns))
        _bass_rust.insert_act_table_loads(self, doctored)

P = 128
D = 256
H = 8
DH = 32
SCALE = 1.0 / float(np.sqrt(DH))
N_CORES = 8
NEG = -1e9

F32 = mybir.dt.float32
BF16 = mybir.dt.bfloat16

# activation dtype switch ("f32" or "bf16")
DT_ACT_NAME = "bf16"


def _dt_act():
    return BF16 if DT_ACT_NAME == "bf16" else F32


def _np_act():
    return ml_dtypes.bfloat16 if DT_ACT_NAME == "bf16" else np.float32


# ---------------------------------------------------------------------------
# weight packing layout (shared between host packer and device program)
# ---------------------------------------------------------------------------
# W_all [128, n_wcols] (dt_act): matmul weight chunks, 128 cols each.
#   chunk_col(base, k, j) = base + k*(2*128) + j*128   (k-outer, j-inner)
#   lin_W  at base 0                      (4 chunks)
#   Wq[i]  at 512 + i*2048 + 0
#   Wk[i]  at 512 + i*2048 + 512
#   Wv[i]  at 512 + i*2048 + 1024
#   Wo[i]  at 512 + i*2048 + 1536
N_WCOLS = 512 + 2 * 2048

LIN_BASE = 0


def w_base(i, which):
    return 512 + i * 2048 + {"q": 0, "k": 512, "v": 1024, "o": 1536}[which]


# C_all [128, n_ccols] f32: per-feature columns (partition = feature within
# d-tile j). col index helpers:
#   0,1   lin_b (j=0,1)
#   2,3   lin_g
#   4,5   lin_beta
#   6+i*12 + [0,1]=bq, [2,3]=bk, [4,5]=bv, [6,7]=bo, [8,9]=ln_g, [10,11]=ln_b
#   30..30+T  maskbias columns (per key-tile)
def c_lin(which, j):
    return {"b": 0, "g": 2, "beta": 4}[which] + j


def c_blk(i, which, j):
    return 6 + i * 12 + {"q": 0, "k": 2, "v": 4, "o": 6, "g": 8, "beta": 10}[which] + j


C_FIXED = 30


# ---------------------------------------------------------------------------
# device program
# ---------------------------------------------------------------------------
def build_program(slot_ts, trivial_ln, trivial_b):
    """slot_ts: tuple of per-slot tile counts (shared across cores).
    trivial_ln: all LN gains are 1 and shifts 0.
    trivial_b: all linear-layer biases are zero (enables pair-merged ops).
    """
    dt = _dt_act()
    T = int(sum(slot_ts))
    NC = T * P  # padded node count per core
    CHW = 512  # chunk width for the n dimension
    NCH = [(c0, min(CHW, NC - c0)) for c0 in range(0, NC, CHW)]  # n chunks

    nc = _Bacc()
    featT = nc.declare_dram_parameter("featT", [P, 2, NC], dt, isOutput=False)
    wall = nc.declare_dram_parameter("wall", [P, N_WCOLS], dt, isOutput=False)
    cons = nc.declare_dram_parameter("cons", [P, C_FIXED + T], F32, isOutput=False)
    outT = nc.declare_dram_parameter("outT", [P, 2, NC], F32, isOutput=True)

    with tile.TileContext(nc) as tc:
        with (
            tc.tile_pool(name="wp", bufs=1) as wp,
            tc.tile_pool(name="xp", bufs=1) as xp,
            tc.tile_pool(name="pp", bufs=max(4, 2 * max(slot_ts))) as pp,
            tc.tile_pool(name="rows", bufs=2) as rows,
            tc.tile_pool(name="psA", bufs=2, space="PSUM") as psA,
            tc.tile_pool(name="psB", bufs=1, space="PSUM") as psB,
        ):
            # psA: tag "work" = 1-bank tiles (bufs=2 -> 2 banks)
            #      tag "work2" = 2-bank pair tiles (bufs=2 -> 4 banks)
            # psB: tag "oT" = one 2-bank pair tile -> 2 banks. total 8.
            w_lin = wp.tile([P, 512], dt, tag="w_lin")
            w_blk = [wp.tile([P, 2048], dt, tag=f"w_blk{i}", name=f"w_blk{i}")
                     for i in range(2)]
            c_sb = wp.tile([P, C_FIXED + T], F32, tag="c")
            nc.sync.dma_start(c_sb[:], cons[:])
            nc.sync.dma_start(w_lin[:], wall[:, 0:512])
            # big weight loads go on other queues so x0/w_lin aren't stuck
            # behind them and block0 can start immediately
            nc.scalar.dma_start(w_blk[0][:], wall[:, 512:2560])
            nc.gpsimd.dma_start(w_blk[1][:], wall[:, 2560:4608])

            def w_tile_of(base):
                if base < 512:
                    return w_lin, base
                i = (base - 512) // 2048
                return w_blk[i], (base - 512) % 2048

            x0 = xp.tile([P, 2, NC], dt, tag="x0", name="x0")
            nc.sync.dma_start(x0[:], featT[:])

            # constants
            ones32 = wp.tile([P, 32], BF16, tag="ones32")
            nc.vector.memset(ones32, 1.0)
            c256 = wp.tile([P, 1], dt, tag="c256")
            nc.vector.memset(c256, 1.0 / 256.0)
            ones_row = wp.tile([1, P], dt, tag="ones_row")
            nc.vector.memset(ones_row, 1.0)
            eps_row = wp.tile([1, 1], F32, tag="eps_row")
            nc.vector.memset(eps_row, 1e-5)

            def wcol(base, k, j, width=P):
                wt, rel = w_tile_of(base)
                c0 = rel + k * 256 + j * 128
                return wt[:, c0 : c0 + width]

            def ccol(idx):
                return c_sb[:, idx : idx + 1]

            def r32(ap):
                # float32r: same bits as f32, single-pass PE mode (vs the
                # 2-pass LOW_HIGH fp32 lowering)
                return ap.bitcast(mybir.dt.float32r) if ap.dtype == F32 else ap

            def rep2(ap):
                # repeat a [P, w] AP along a stride-0 middle dim -> [P, 2, w]
                return bass.AP(tensor=ap.tensor, offset=ap.offset,
                               ap=[list(ap.ap[0]), [0, 2]] + [list(a) for a in ap.ap[1:]])

            def proj_pair(src, base, bias_idx, out_dt, func=None, scale=1.0,
                          eng="act"):
                """pair projection: out[:, j, n] = func(W_j.T @ src + b_j)."""
                out = xp.tile([P, 2, NC], out_dt, tag=f"pj{base}",
                              name=f"pj{base}")
                func = func or mybir.ActivationFunctionType.Identity
                for c0, cw in NCH:
                    ps = psA.tile([P, 2, cw], F32, tag="work2", name="pp2")
                    for j in range(2):
                        for k in range(2):
                            nc.tensor.matmul(
                                ps[:, j, :],
                                r32(wcol(base, k, j)),
                                r32(src[:, k, c0 : c0 + cw]),
                                start=(k == 0),
                                stop=(k == 1),
                            )
                    dst = out[:, :, c0 : c0 + cw]
                    if trivial_b:
                        if eng == "dve":
                            nc.vector.tensor_copy(dst, ps)
                        else:
                            nc.scalar.activation(dst, ps, func, scale=scale)
                    else:
                        for j in range(2):
                            if eng == "dve":
                                nc.vector.tensor_scalar_add(
                                    out[:, j, c0 : c0 + cw], ps[:, j, :],
                                    ccol(bias_idx + j))
                            else:
                                nc.scalar.activation(
                                    out[:, j, c0 : c0 + cw], ps[:, j, :], func,
                                    bias=ccol(bias_idx + j), scale=scale)
                return out

            def layernorm_pair(y, gcol, bcol, out_dt):
                """LayerNorm over the feature dim (partition axis, 2 tiles)."""
                out = xp.tile([P, 2, NC], out_dt, tag="lnout", name="lnout")
                sq = xp.tile([P, 2, NC], dt, tag="sq", name="sq")
                nc.vector.tensor_mul(sq, y, y)
                for c0, cw in NCH:
                    stats = psA.tile([33, cw], F32, tag="work", name="stats")
                    for k in range(2):
                        nc.tensor.matmul(
                            stats[0:1, :], r32(c256),
                            r32(y[:, k, c0 : c0 + cw]),
                            start=(k == 0), stop=(k == 1),
                        )
                    for k in range(2):
                        nc.tensor.matmul(
                            stats[32:33, :], r32(c256),
                            r32(sq[:, k, c0 : c0 + cw]),
                            start=(k == 0), stop=(k == 1),
                            tile_position=(0, 32),
                        )
                    # rows: var = E[y^2] - mean^2 ; rstd = 1/sqrt(var+eps)
                    m2 = rows.tile([1, cw], F32, tag="m2")
                    nc.scalar.activation(
                        m2, stats[0:1, :], mybir.ActivationFunctionType.Square
                    )
                    var = rows.tile([1, cw], F32, tag="var")
                    nc.vector.scalar_tensor_tensor(
                        var, m2, -1.0, stats[32:33, :],
                        op0=mybir.AluOpType.mult, op1=mybir.AluOpType.add,
                    )
                    # rstd = exp(-0.5*ln(var+eps)) — Ln and Exp live in one
                    # ACT table set (Sqrt would force a table switch)
                    lnv = rows.tile([1, cw], F32, tag="lnv")
                    nc.scalar.activation(
                        lnv, var, mybir.ActivationFunctionType.Ln,
                        bias=eps_row[:]
                    )
                    rstd = rows.tile([1, cw], dt, tag="rstd")
                    nc.scalar.activation(
                        rstd, lnv, mybir.ActivationFunctionType.Exp, scale=-0.5
                    )
                    ms = rows.tile([1, cw], dt, tag="ms")
                    nc.vector.tensor_mul(ms, stats[0:1, :], rstd)
                    # broadcast both rows to 128 partitions via K=1 matmuls
                    sbm = psA.tile([P, 2, cw], F32, tag="work2", name="sbm")
                    nc.tensor.matmul(sbm[:, 0, :], r32(ones_row), r32(rstd),
                                     start=True, stop=True)
                    nc.tensor.matmul(sbm[:, 1, :], r32(ones_row), r32(ms),
                                     start=True, stop=True)
                    t1 = xp.tile([P, 2, cw], F32, tag="ln_t1", name="ln_t1")
                    nc.vector.tensor_mul(
                        t1, y[:, :, c0 : c0 + cw], rep2(sbm[:, 0, :]))
                    dst = out[:, :, c0 : c0 + cw]
                    if trivial_ln:
                        nc.vector.tensor_sub(dst, t1, rep2(sbm[:, 1, :]))
                    else:
                        t2 = xp.tile([P, 2, cw], F32, tag="ln_t2", name="ln_t2")
                        nc.vector.tensor_sub(t2, t1, rep2(sbm[:, 1, :]))
                        for k in range(2):
                            nc.vector.tensor_scalar(
                                out[:, k, c0 : c0 + cw], t2[:, k, :],
                                ccol(gcol + k), ccol(bcol + k),
                                op0=mybir.AluOpType.mult,
                                op1=mybir.AluOpType.add,
                            )
                return out

            def attention_block(i, hp):
                """one MHA block on the pair tile hp; returns new x pair."""
                qT = proj_pair(hp, w_base(i, "q"), c_blk(i, "q", 0), BF16)
                kT = proj_pair(hp, w_base(i, "k"), c_blk(i, "k", 0), BF16,
                               eng="dve")
                # matmul operands must sit at partition 0 on this stack:
                # shift each head's 32 rows down via SBUF->SBUF DMA, one
                # tile per head so scores wait only on their own DMA
                q_h = [xp.tile([32, NC], BF16, tag=f"q_h{h}", name=f"q_h{h}")
                       for h in range(H)]
                k_h = [xp.tile([32, NC], BF16, tag=f"k_h{h}", name=f"k_h{h}")
                       for h in range(H)]
                engs = [nc.sync, nc.gpsimd, nc.scalar]
                for h in range(H):
                    b, hh = divmod(h, 4)
                    engs[h % 3].dma_start(
                        q_h[h][:], qT[32 * hh : 32 * hh + 32, b, :])
                    engs[(h + 1) % 3].dma_start(
                        k_h[h][:], kT[32 * hh : 32 * hh + 32, b, :])
                # v in node layout: v[n_tile, d] = hT_chunk.T @ Wv_chunk
                v_sb = xp.tile([P, T, 256], BF16, tag="v_all")
                for t2i in range(0, T, 2):
                    npair = min(2, T - t2i)
                    vp = psA.tile([P, npair, 256], F32, tag="work", name="vp")
                    for tt in range(npair):
                        t = t2i + tt
                        for k in range(2):
                            nc.tensor.matmul(
                                vp[:, tt, :],
                                r32(hp[:, k, t * P : (t + 1) * P]),
                                r32(wcol(w_base(i, "v"), k, 0, width=256)),
                                start=(k == 0),
                                stop=(k == 1),
                            )
                    nc.scalar.activation(
                        v_sb[:, t2i : t2i + npair, :], vp,
                        mybir.ActivationFunctionType.Copy)
                # attention per slot
                o_sb = xp.tile([P, 2, NC], dt, tag="o_sb", name="o_sb")
                all_unit = all(t == 1 for t in slot_ts)
                for c0, cw in NCH:
                    oT_ps = psB.tile([P, 2, cw], F32, tag="oT", name="oT")
                    dn_ps = None
                    if not all_unit:
                        dn_ps = psA.tile([P, 2, cw], F32, tag="work2",
                                         name="dn")
                    # pT for the whole chunk in one tile: exp writes slot
                    # blocks, the denominator is one matmul per (bank, head)
                    ctiles = cw // P
                    pT_all = pp.tile([P, ctiles, 2, 4, P], BF16, tag="pTall",
                                     name="pT_all")
                    t_off = 0
                    q_off = 0
                    for s, ts_s in enumerate(slot_ts):
                        for qc0 in range(q_off, q_off + ts_s * P, P):
                            if qc0 < c0 or qc0 >= c0 + cw:
                                continue
                            qrel = qc0 - c0
                            qi = qrel // P
                            pT = {}
                            for kt in range(ts_s):
                                ktg = t_off + kt
                                if all_unit and ts_s == 1:
                                    p_t = pT_all[:, qi, :, :, :]
                                else:
                                    p_t = pp.tile([P, 2, 4, P], BF16,
                                                  tag="pT", name="p_t")
                                stp = psA.tile([P, 2, 4, P], F32, tag="work2",
                                               name="stp")
                                for b in range(2):
                                    for hh in range(4):
                                        nc.tensor.matmul(
                                            stp[:, b, hh, :],
                                            k_h[4 * b + hh][:,
                                                ktg * P : (ktg + 1) * P],
                                            q_h[4 * b + hh][:, qc0 : qc0 + P],
                                            start=True,
                                            stop=True,
                                        )
                                # one exp over both head-banks (2-bank read)
                                nc.scalar.activation(
                                    p_t,
                                    stp,
                                    mybir.ActivationFunctionType.Exp,
                                    scale=SCALE,
                                    bias=ccol(C_FIXED + ktg),
                                )
                                pT[kt] = p_t
                            # PV accumulation, one closed psum group per
                            # (bank, head) at a time
                            for b in range(2):
                                for hh in range(4):
                                    for kt in range(ts_s):
                                        nc.tensor.matmul(
                                            oT_ps[32 * hh : 32 * hh + 32,
                                                  b, qrel : qrel + P],
                                            v_sb[:, t_off + kt,
                                                 (4 * b + hh) * 32 : (4 * b + hh) * 32 + 32],
                                            pT[kt][:, b, hh, :],
                                            start=(kt == 0),
                                            stop=(kt == ts_s - 1),
                                            tile_position=(0, 32 * hh),
                                        )
                                    if not all_unit:
                                        for kt in range(ts_s):
                                            nc.tensor.matmul(
                                                dn_ps[32 * hh : 32 * hh + 32,
                                                      b, qrel : qrel + P],
                                                ones32,
                                                pT[kt][:, b, hh, :],
                                                start=(kt == 0),
                                                stop=(kt == ts_s - 1),
                                                tile_position=(0, 32 * hh),
                                            )
                        t_off += ts_s
                        q_off += ts_s * P
                    if all_unit:
                        # merged denominators: each slot's keys live on the
                        # partition axis of its own pT block
                        dn_ps = psA.tile([P, 2, cw], F32, tag="work2",
                                         name="dn")
                        for b in range(2):
                            for hh in range(4):
                                nc.tensor.matmul(
                                    dn_ps[32 * hh : 32 * hh + 32, b, :],
                                    ones32,
                                    pT_all[:, :, b, hh, :],
                                    start=True,
                                    stop=True,
                                    tile_position=(0, 32 * hh),
                                )
                    # normalize: o = oT * (1/dn), fused with PSUM->SBUF copy
                    r_sb = pp.tile([P, 2, cw], F32, tag="r", name="r_sb")
                    nc.vector.reciprocal_approx_fast(out=r_sb, in_=dn_ps)
                    nc.vector.tensor_mul(o_sb[:, :, c0 : c0 + cw], oT_ps, r_sb)
                # output projection + residual
                y = xp.tile([P, 2, NC], dt, tag="y", name="y")
                for c0, cw in NCH:
                    zp = psA.tile([P, 2, cw], F32, tag="work2", name="zp")
                    for j in range(2):
                        for k in range(2):
                            nc.tensor.matmul(
                                zp[:, j, :],
                                r32(wcol(w_base(i, "o"), k, j)),
                                r32(o_sb[:, k, c0 : c0 + cw]),
                                start=(k == 0),
                                stop=(k == 1),
                            )
                    if trivial_b:
                        nc.vector.tensor_add(
                            y[:, :, c0 : c0 + cw], zp, hp[:, :, c0 : c0 + cw])
                    else:
                        for j in range(2):
                            nc.vector.scalar_tensor_tensor(
                                y[:, j, c0 : c0 + cw], zp[:, j, :],
                                ccol(c_blk(i, "o", j)),
                                hp[:, j, c0 : c0 + cw],
                                op0=mybir.AluOpType.add,
                                op1=mybir.AluOpType.add,
                            )
                out_dt = F32 if i == 1 else dt
                return layernorm_pair(
                    y, c_blk(i, "g", 0), c_blk(i, "beta", 0), out_dt
                )

            # block 0 pre-layer: LN(tanh(x @ lin_W + lin_b)) * g + beta.
            # tanh = 1 - 2/(e^{2u}+1) via Exp keeps ACT on one table set
            # (the packed lin bias column is pre-doubled on the host)
            e0 = proj_pair(x0, LIN_BASE, c_lin("b", 0), F32,
                           func=mybir.ActivationFunctionType.Exp, scale=2.0)
            ep1 = xp.tile([P, 2, NC], F32, tag="ep1", name="ep1")
            nc.vector.tensor_scalar_add(ep1, e0, 1.0)
            rp = xp.tile([P, 2, NC], F32, tag="rp", name="rp")
            nc.vector.reciprocal_approx_fast(out=rp, in_=ep1)
            t0 = xp.tile([P, 2, NC], dt, tag="t0", name="t0")
            nc.vector.tensor_scalar(
                t0, rp, -2.0, 1.0,
                op0=mybir.AluOpType.mult, op1=mybir.AluOpType.add,
            )
            h0 = layernorm_pair(t0, c_lin("g", 0), c_lin("beta", 0), dt)
            x1 = attention_block(0, h0)
            x2 = attention_block(1, x1)
            nc.sync.dma_start(outT[:], x2[:])

    nc.finalize()
    return nc


# ---------------------------------------------------------------------------
# host side
# ---------------------------------------------------------------------------
_prog_cache = {}
_last_results = None


def _get_program(slot_ts, trivial_ln, trivial_b):
    key = (tuple(slot_ts), trivial_ln, trivial_b, DT_ACT_NAME)
    if key not in _prog_cache:
        _prog_cache[key] = build_program(tuple(slot_ts), trivial_ln, trivial_b)
    return _prog_cache[key]


def _segments(ordering):
    """contiguous runs of equal values in sorted ordering -> (start, len)."""
    n = ordering.shape[0]
    change = np.nonzero(np.diff(ordering))[0] + 1
    starts = np.concatenate([[0], change])
    lens = np.diff(np.concatenate([starts, [n]]))
    return list(zip(starts.tolist(), lens.tolist()))


def kernel(
    feat, ordering, lin_W, lin_b, lin_g, lin_beta,
    Wq, Wk, Wv, bq, bk, bv, Wo, bo, ln_g, ln_b,
):
    feat = np.asarray(feat, np.float32)
    ordering = np.asarray(ordering)
    N = feat.shape[0]
    np_act = _np_act()

    perm = np.argsort(ordering, kind="stable")
    segs = _segments(np.asarray(ordering)[perm])

    # deal segments (sorted by length desc) snake-wise to cores
    order = sorted(range(len(segs)), key=lambda i: -segs[i][1])
    core_slots = [[] for _ in range(N_CORES)]
    for r, si in enumerate(order):
        c = r % (2 * N_CORES)
        c = c if c < N_CORES else 2 * N_CORES - 1 - c
        core_slots[c].append(si)
    S = max(len(cs) for cs in core_slots)
    # per-slot tile count = max over cores of the slot's segment size
    slot_ts = []
    for k in range(S):
        mx = 1
        for c in range(N_CORES):
            if k < len(core_slots[c]):
                mx = max(mx, (segs[core_slots[c][k]][1] + P - 1) // P)
        slot_ts.append(mx)
    T = sum(slot_ts)
    NC = T * P

    trivial_ln = bool(
        np.all(np.asarray(ln_g) == 1) and np.all(np.asarray(ln_b) == 0)
        and np.all(np.asarray(lin_g) == 1) and np.all(np.asarray(lin_beta) == 0)
    )
    # softmax rows sum to 1, so attn @ (v + 1*bv^T) = attn@v + 1*bv^T; the
    # per-block value bias folds exactly into the output-projection bias:
    # bo_eff = bo + bv @ Wo
    bo = np.asarray(bo, np.float32) + np.einsum(
        "id,idj->ij", np.asarray(bv, np.float32), np.asarray(Wo, np.float32)
    )

    trivial_b = bool(
        np.all(np.asarray(lin_b) == 0) and np.all(np.asarray(bq) == 0)
        and np.all(np.asarray(bk) == 0) and np.all(np.asarray(bo) == 0)
    )
    nc = _get_program(slot_ts, trivial_ln, trivial_b)

    # ---- pack weights ----
    wallp = np.zeros((P, N_WCOLS), np.float32)

    def put_w(base, W):
        W = np.asarray(W, np.float32)
        for k in range(2):
            for j in range(2):
                c0 = base + k * 256 + j * 128
                wallp[:, c0 : c0 + 128] = W[k * 128 : (k + 1) * 128,
                                            j * 128 : (j + 1) * 128]

    put_w(LIN_BASE, lin_W)
    for i in range(2):
        put_w(w_base(i, "q"), np.asarray(Wq)[i])
        put_w(w_base(i, "k"), np.asarray(Wk)[i])
        put_w(w_base(i, "v"), np.asarray(Wv)[i])
        put_w(w_base(i, "o"), np.asarray(Wo)[i])
    wallp = wallp.astype(np_act)

    consp = np.zeros((P, C_FIXED + T), np.float32)

    def put_c(idx, vec):
        vec = np.asarray(vec, np.float32)
        consp[:, idx] = vec[:128]
        consp[:, idx + 1] = vec[128:]

    # lin bias is consumed inside Exp(2*u + 2*b) for the tanh-via-exp path
    put_c(c_lin("b", 0), np.asarray(lin_b, np.float32) * 2.0)
    put_c(c_lin("g", 0), lin_g)
    put_c(c_lin("beta", 0), lin_beta)
    for i in range(2):
        put_c(c_blk(i, "q", 0), np.asarray(bq)[i])
        put_c(c_blk(i, "k", 0), np.asarray(bk)[i])
        put_c(c_blk(i, "v", 0), np.asarray(bv)[i] * 0)
        put_c(c_blk(i, "o", 0), np.asarray(bo)[i])
        put_c(c_blk(i, "g", 0), np.asarray(ln_g)[i])
        put_c(c_blk(i, "beta", 0), np.asarray(ln_b)[i])

    # ---- per-core data ----
    feat_sorted = feat[perm]
    in_maps = []
    core_meta = []  # (slot k, seg_start, seg_len, node_offset in padded layout)
    for c in range(N_CORES):
        fT = np.zeros((NC, 256), np.float32)
        mb = np.full((NC,), NEG, np.float32)
        meta = []
        off = 0
        for k in range(S):
            if k < len(core_slots[c]):
                st, ln = segs[core_slots[c][k]]
                fT[off : off + ln] = feat_sorted[st : st + ln]
                mb[off : off + ln] = 0.0
                meta.append((st, ln, off))
            off += slot_ts[k] * P
        cons_c = consp.copy()
        cons_c[:, C_FIXED : C_FIXED + T] = mb.reshape(T, P).T
        featT_c = np.ascontiguousarray(
            fT.T.reshape(2, P, NC).transpose(1, 0, 2)
        ).astype(np_act)
        in_maps.append({"featT": featT_c, "wall": wallp, "cons": cons_c})
        core_meta.append(meta)

    res = run_bass_kernel_spmd(nc, in_maps, list(range(N_CORES)))
    global _last_results
    _last_results = res

    out = np.empty((N, 256), np.float32)
    for c in range(N_CORES):
        oT = np.asarray(res.results[c]["outT"], np.float32)  # [128, 2, NC]
        o_nodes = oT.transpose(1, 0, 2).reshape(256, NC).T  # [NC, 256]
        for st, ln, off in core_meta[c]:
            out[perm[st : st + ln]] = o_nodes[off : off + ln]
    return out



# revision 8
# speedup vs baseline: 1.1889x; 1.1889x over previous
"""Trainium2 Bass kernel for BatchedStarNetAttentionBlock.

Strategy: data-parallel over ordering segments (attention is block-diagonal,
never crosses segment boundaries). Each of the 8 cores gets a subset of
segments, padded to a shared static structure so one SPMD program serves all
cores. No collectives.

On-device layout: activations are kept transposed, xT[d, n] with the feature
dim on partitions (2 tiles of 128), so every linear layer is a natural
matmul (lhsT = weight chunk [k,j], rhs = xT chunk [k,n]). Scores are computed
directly in transposed form S.T = kT.T @ qT with the per-head 32-row slices
of kT/qT addressed in place at partition offsets 32h via PE row-group
tile_position (no SBUF shuffle, 4 heads run concurrently in the PE array).
PV is col-tiled by head so attention output lands as oT[d, n] in PSUM.
Denominators come from ones-matmuls writing a row-replicated bank with the
same layout as oT.

LayerNorm uses replicated-mean matmuls (lhsT = [128,128] tile of 1/256) so
the mean arrives in PSUM already broadcast across partitions: no row ops and
no separate broadcast matmuls. var = E[(y-m)^2] via a second replicated
matmul over z^2; rstd = exp(-0.5*ln(var+eps)) on full-width tiles. The mean
of a post-attention residual y = Wo.T o + h is computed before y itself
exists, via lhsT tiles of row-means of Wo (host-packed) plus 1/256 against
h, overlapping the O-projection.

Block0's Tanh uses the ACT Tanh entry directly (activation-table planner is
doctored so table set 0 claims only Tanh; everything else lives in the
natural_log_exp set -> exactly two table loads, both off critical path).

Extra touches: dummy warm-up matmuls keep the PE HAM clock-gate from running
the first real matmuls at 1.2 GHz; input DMAs are split across the three
DMA-capable queues in consumption order; elementwise work is spread across
DVE / ACT / GpSimd.
"""

import sys

for _p in ("/opt/trn_rl_repo",):
    if _p not in sys.path:
        sys.path.insert(0, _p)

import numpy as np
import ml_dtypes

import bass_rust as _bass_rust

import concourse.bass as bass
import concourse.tile as tile
from concourse import bacc
from concourse import mybir
from concourse.bass_utils import run_bass_kernel_spmd
from concourse.hw_specs import get_activation_tables


class _Bacc(bacc.Bacc):
    """Bacc whose activation-table planner maps Tanh to table set 0 and every
    other used function (Ln/Exp/Square/Identity/Copy) to the one set that
    contains them all, so the program pays exactly two ACT table loads."""

    def insert_act_table_loads(self):
        has_activation = any(
            isinstance(i, mybir.InstActivation)
            for b in self.main_func.blocks
            for i in b.instructions
        )
        if not has_activation:
            return
        tables = list(get_activation_tables(self.m.arch).items())
        # The planner emits act_func_set_id = position in this list, so
        # positions must stay aligned with act_info.json. Narrow the match
        # sets instead: position 0 claims only tanh; other sets before
        # natural_log_exp_and_others claim nothing; so exp/ln/square/
        # identity/copy all resolve to the one set that has them all.
        pref = "natural_log_exp_and_others"
        TANH = mybir.ActivationFunctionType.Tanh
        doctored = []
        seen_pref = False
        for name, fns in tables:
            if name == pref:
                seen_pref = True
                doctored.append((name, fns))
            elif not seen_pref:
                doctored.append((name, {TANH} & fns))
            else:
                doctored.append((name, fns))
        _bass_rust.insert_act_table_loads(self, doctored)

P = 128
D = 256
H = 8
DH = 32
SCALE = 1.0 / float(np.sqrt(DH))
N_CORES = 8
NEG = -1e9

F32 = mybir.dt.float32
BF16 = mybir.dt.bfloat16

# activation dtype switch ("f32" or "bf16")
DT_ACT_NAME = "bf16"

# score matmuls read qT/kT head slices in place at partition offsets 32h
# (PE row groups); False falls back to SBUF->SBUF shuffles to partition 0
DIRECT_SCORES = True


def _dt_act():
    return BF16 if DT_ACT_NAME == "bf16" else F32


def _np_act():
    return ml_dtypes.bfloat16 if DT_ACT_NAME == "bf16" else np.float32


# ---------------------------------------------------------------------------
# weight packing layout (shared between host packer and device program)
# ---------------------------------------------------------------------------
# W_all [128, n_wcols] (dt_act): matmul weight chunks, 128 cols each.
#   chunk_col(base, k, j) = base + k*(2*128) + j*128   (k-outer, j-inner)
#   lin_W  at base 0                      (4 chunks)
#   Wq[i]  at 512 + i*2048 + 0
#   Wk[i]  at 512 + i*2048 + 512
#   Wv[i]  at 512 + i*2048 + 1024
#   Wo[i]  at 512 + i*2048 + 1536
#   wo_rm[i][k] at 4608 + i*256 + k*128 : row-means of Wo[i] (over the output
#     dim), replicated across the 128 columns -> replicated-mean lhsT tiles.
N_WCOLS = 512 + 2 * 2048 + 512

LIN_BASE = 0
WO_RM = 512 + 2 * 2048


def w_base(i, which):
    return 512 + i * 2048 + {"q": 0, "k": 512, "v": 1024, "o": 1536}[which]


# C_all [128, n_ccols] f32: per-feature columns (partition = feature within
# d-tile j). col index helpers:
#   0,1   lin_b (j=0,1)
#   2,3   lin_g
#   4,5   lin_beta
#   6+i*12 + [0,1]=bq, [2,3]=bk, [4,5]=bv, [6,7]=bo, [8,9]=ln_g, [10,11]=ln_b
#   30..30+T  maskbias columns (per key-tile)
def c_lin(which, j):
    return {"b": 0, "g": 2, "beta": 4}[which] + j


def c_blk(i, which, j):
    return 6 + i * 12 + {"q": 0, "k": 2, "v": 4, "o": 6, "g": 8, "beta": 10}[which] + j


C_FIXED = 30


# ---------------------------------------------------------------------------
# device program
# ---------------------------------------------------------------------------
def build_program(slot_ts, trivial_ln, trivial_b):
    """slot_ts: tuple of per-slot tile counts (shared across cores).
    trivial_ln: all LN gains are 1 and shifts 0.
    trivial_b: all linear-layer biases are zero (enables pair-merged ops).
    """
    dt = _dt_act()
    T = int(sum(slot_ts))
    NC = T * P  # padded node count per core
    CHW = 256  # chunk width for the n dimension
    NCH = [(c0, min(CHW, NC - c0)) for c0 in range(0, NC, CHW)]  # n chunks

    TANH = mybir.ActivationFunctionType.Tanh
    LN_F = mybir.ActivationFunctionType.Ln
    EXP = mybir.ActivationFunctionType.Exp
    IDENT = mybir.ActivationFunctionType.Identity

    nc = _Bacc()
    featT = nc.declare_dram_parameter("featT", [P, 2, NC], dt, isOutput=False)
    wall = nc.declare_dram_parameter("wall", [P, N_WCOLS], dt, isOutput=False)
    cons = nc.declare_dram_parameter("cons", [P, C_FIXED + T], F32, isOutput=False)
    outT = nc.declare_dram_parameter("outT", [P, 2, NC], F32, isOutput=True)

    with tile.TileContext(nc) as tc:
        with (
            tc.tile_pool(name="wp", bufs=1) as wp,
            tc.tile_pool(name="xp", bufs=1) as xp,
            tc.tile_pool(name="pp", bufs=2) as pp,
            tc.tile_pool(name="psA", bufs=2, space="PSUM") as psA,
            tc.tile_pool(name="psB", bufs=2, space="PSUM") as psB,
        ):
            # PSUM: psA tag "work" = 1-bank tiles (x2), tag "work2" = 2-bank
            # tiles (x2); psB tag "oT" 1-bank (x2). Total 8 banks.
            w_lin = wp.tile([P, 512], dt, tag="w_lin")
            w_blk = [wp.tile([P, 2048], dt, tag=f"w_blk{i}", name=f"w_blk{i}")
                     for i in range(2)]
            worm = wp.tile([P, 512], dt, tag="worm")
            c_sb = wp.tile([P, C_FIXED + T], F32, tag="c")
            x0 = xp.tile([P, 2, NC], dt, tag="x0", name="x0")

            # input DMAs, split across the three DMA-capable queues in
            # consumption order: lin weights + x0 first, then block0's
            # attention weights, then block1's.
            h_nc = NC // 2
            nc.sync.dma_start(x0[:, :, 0:h_nc], featT[:, :, 0:h_nc])
            nc.scalar.dma_start(w_lin[:], wall[:, 0:512])
            nc.sync.dma_start(x0[:, :, h_nc:NC], featT[:, :, h_nc:NC])
            nc.gpsimd.dma_start(c_sb[:], cons[:])
            nc.gpsimd.dma_start(worm[:], wall[:, WO_RM : WO_RM + 512])
            nc.scalar.dma_start(w_blk[0][:, 0:1024], wall[:, 512:1536])
            nc.gpsimd.dma_start(w_blk[0][:, 1024:2048], wall[:, 1536:2560])
            nc.sync.dma_start(w_blk[1][:], wall[:, 2560:4608])

            def w_tile_of(base):
                if base < 512:
                    return w_lin, base
                if base >= WO_RM:
                    return worm, base - WO_RM
                i = (base - 512) // 2048
                return w_blk[i], (base - 512) % 2048

            # constants
            ones32 = wp.tile([P, 32], BF16, tag="ones32")
            nc.vector.memset(ones32, 1.0)
            c256rep = wp.tile([P, P], BF16, tag="c256rep")
            nc.vector.memset(c256rep, 1.0 / 256.0)
            eps_col = wp.tile([P, 1], F32, tag="eps_col")
            nc.vector.memset(eps_col, 1e-5)

            def repf(ap, n):
                # repeat a [P, w] AP along a stride-0 middle dim -> [P, n, w]
                return bass.AP(tensor=ap.tensor, offset=ap.offset,
                               ap=[list(ap.ap[0]), [0, n]] + [list(a) for a in ap.ap[1:]])

            # HAM warm-up: keep the PE busy for ~4us while inputs stream in,
            # so the clock gate is released before the first real matmul.
            hamrhs = wp.tile([P, 512], BF16, tag="hamrhs")
            nc.vector.memset(hamrhs, 0.5)
            ham = psA.tile([P, 512], F32, tag="work", name="ham")
            for _ in range(8):
                nc.tensor.matmul(ham[:], c256rep[:], hamrhs[:],
                                 start=True, stop=True)

            def wcol(base, k, j, width=P):
                wt, rel = w_tile_of(base)
                c0 = rel + k * 256 + j * 128
                return wt[:, c0 : c0 + width]

            def wormcol(i, k):
                return worm[:, i * 256 + k * 128 : i * 256 + k * 128 + P]

            def ccol(idx):
                return c_sb[:, idx : idx + 1]

            def r32(ap):
                # float32r: same bits as f32, single-pass PE mode (vs the
                # 2-pass LOW_HIGH fp32 lowering)
                return ap.bitcast(mybir.dt.float32r) if ap.dtype == F32 else ap

            def rep2(ap):
                # repeat a [P, w] AP along a stride-0 middle dim -> [P, 2, w]
                return repf(ap, 2)

            def proj_pair(src, base, bias_idx, out_dt, func=None, scale=1.0,
                          eng="act", tag=None):
                """pair projection: out[:, j, n] = func(W_j.T @ src + b_j)."""
                out = xp.tile([P, 2, NC], out_dt, tag=tag or f"pj{base}",
                              name=tag or f"pj{base}")
                func = func or IDENT
                for c0, cw in NCH:
                    ps = psA.tile([P, 2, cw], F32, tag="work", name="pp2")
                    for j in range(2):
                        for k in range(2):
                            nc.tensor.matmul(
                                ps[:, j, :],
                                r32(wcol(base, k, j)),
                                r32(src[:, k, c0 : c0 + cw]),
                                start=(k == 0),
                                stop=(k == 1),
                            )
                    dst = out[:, :, c0 : c0 + cw]
                    if trivial_b:
                        if eng == "dve":
                            nc.vector.tensor_copy(dst, ps)
                        else:
                            nc.scalar.activation(dst, ps, func, scale=scale)
                    else:
                        for j in range(2):
                            if eng == "dve":
                                nc.vector.tensor_scalar_add(
                                    out[:, j, c0 : c0 + cw], ps[:, j, :],
                                    ccol(bias_idx + j))
                            else:
                                nc.scalar.activation(
                                    out[:, j, c0 : c0 + cw], ps[:, j, :], func,
                                    bias=ccol(bias_idx + j), scale=scale)
                return out

            def layernorm_pair(y, gcol, bcol, out_dt, tag, mean_terms=None):
                """LayerNorm over the feature dim (partition axis, 2 tiles).

                mean_terms: optional list of (lhsT_fn(k) -> AP, src_pair)
                whose accumulated matmuls give the replicated mean of y;
                defaults to [(1/256 tile, y)]. Lets callers overlap the mean
                with producing y itself.
                """
                out = xp.tile([P, 2, NC], out_dt, tag=tag, name=tag)
                z = xp.tile([P, 2, NC], dt, tag=f"{tag}_z", name=f"{tag}_z")
                sq = xp.tile([P, 2, NC], dt, tag=f"{tag}_sq", name=f"{tag}_sq")
                if mean_terms is None:
                    mean_terms = [(lambda k: c256rep[:], y)]
                for c0, cw in NCH:
                    m_b = psA.tile([P, cw], F32, tag="work", name="m_b")
                    n_mm = 2 * len(mean_terms)
                    idx = 0
                    for lf, src in mean_terms:
                        for k in range(2):
                            nc.tensor.matmul(
                                m_b[:], r32(lf(k)),
                                r32(src[:, k, c0 : c0 + cw]),
                                start=(idx == 0), stop=(idx == n_mm - 1),
                            )
                            idx += 1
                    zc = z[:, :, c0 : c0 + cw]
                    nc.vector.tensor_sub(zc, y[:, :, c0 : c0 + cw], rep2(m_b))
                    sqc = sq[:, :, c0 : c0 + cw]
                    nc.gpsimd.tensor_mul(sqc, zc, zc)
                    var_b = psA.tile([P, cw], F32, tag="work", name="var_b")
                    for k in range(2):
                        nc.tensor.matmul(
                            var_b[:], r32(c256rep[:]),
                            r32(sq[:, k, c0 : c0 + cw]),
                            start=(k == 0), stop=(k == 1),
                        )
                    # rstd = exp(-0.5*ln(var+eps)) on the replicated tile
                    lnv = pp.tile([P, cw], F32, tag="lnv", name="lnv")
                    nc.scalar.activation(lnv, var_b, LN_F, bias=eps_col[:])
                    rstd = pp.tile([P, cw], dt, tag="rstd", name="rstd")
                    nc.scalar.activation(rstd, lnv, EXP, scale=-0.5)
                    dst = out[:, :, c0 : c0 + cw]
                    if trivial_ln:
                        nc.vector.tensor_mul(dst, zc, rep2(rstd))
                    else:
                        t2 = xp.tile([P, 2, cw], dt, tag="ln_t2", name="ln_t2")
                        nc.vector.tensor_mul(t2, zc, rep2(rstd))
                        for k in range(2):
                            nc.vector.tensor_scalar(
                                out[:, k, c0 : c0 + cw], t2[:, k, :],
                                ccol(gcol + k), ccol(bcol + k),
                                op0=mybir.AluOpType.mult,
                                op1=mybir.AluOpType.add,
                            )
                return out

            def attention_block(i, hp):
                """one MHA block on the pair tile hp; returns new x pair."""
                qT = proj_pair(hp, w_base(i, "q"), c_blk(i, "q", 0), BF16,
                               tag=f"qT{i}")
                kT = proj_pair(hp, w_base(i, "k"), c_blk(i, "k", 0), BF16,
                               eng="dve", tag=f"kT{i}")
                # v in node layout: v[n_tile, d] = hT_chunk.T @ Wv_chunk
                v_sb = xp.tile([P, T, 256], BF16, tag=f"v{i}", name=f"v{i}")
                for t2i in range(0, T, 2):
                    npair = min(2, T - t2i)
                    vp = psA.tile([P, npair, 256], F32, tag="work", name="vp")
                    for tt in range(npair):
                        t = t2i + tt
                        for k in range(2):
                            nc.tensor.matmul(
                                vp[:, tt, :],
                                r32(hp[:, k, t * P : (t + 1) * P]),
                                r32(wcol(w_base(i, "v"), k, 0, width=256)),
                                start=(k == 0),
                                stop=(k == 1),
                            )
                    nc.vector.tensor_copy(v_sb[:, t2i : t2i + npair, :], vp)
                if DIRECT_SCORES:
                    def khead(hh, b, cslice):
                        return kT[32 * hh : 32 * hh + 32, b, cslice]

                    def qhead(hh, b, cslice):
                        return qT[32 * hh : 32 * hh + 32, b, cslice]

                    def stile(hh):
                        return (32 * hh, 0)
                else:
                    q_h = [xp.tile([32, NC], BF16, tag=f"q{i}h{h}",
                                   name=f"q{i}h{h}") for h in range(H)]
                    k_h = [xp.tile([32, NC], BF16, tag=f"k{i}h{h}",
                                   name=f"k{i}h{h}") for h in range(H)]
                    engs = [nc.sync, nc.gpsimd, nc.scalar]
                    for h in range(H):
                        b, hh = divmod(h, 4)
                        engs[h % 3].dma_start(
                            q_h[h][:], qT[32 * hh : 32 * hh + 32, b, :])
                        engs[(h + 1) % 3].dma_start(
                            k_h[h][:], kT[32 * hh : 32 * hh + 32, b, :])

                    def khead(hh, b, cslice):
                        return k_h[4 * b + hh][:, cslice]

                    def qhead(hh, b, cslice):
                        return q_h[4 * b + hh][:, cslice]

                    def stile(hh):
                        return (0, 0)
                # attention per slot; scores read qT/kT per-head slices in
                # place at partition offset 32h (PE row groups, 4 heads
                # concurrent in the array)
                o_sb = xp.tile([P, 2, NC], dt, tag=f"o{i}", name=f"o{i}")
                all_unit = all(t == 1 for t in slot_ts)
                for c0, cw in NCH:
                    oT_ps = psB.tile([P, 2, cw], F32, tag="oT", name="oT")
                    dn_ps = None
                    if not all_unit:
                        dn_ps = psA.tile([P, 2, cw], F32, tag="work2",
                                         name="dn")
                    ctiles = cw // P
                    pT_all = pp.tile([P, ctiles, 2, 4, P], BF16, tag="pTall",
                                     name="pT_all")
                    t_off = 0
                    q_off = 0
                    for s, ts_s in enumerate(slot_ts):
                        for qc0 in range(q_off, q_off + ts_s * P, P):
                            if qc0 < c0 or qc0 >= c0 + cw:
                                continue
                            qrel = qc0 - c0
                            qi = qrel // P
                            pT = {}
                            for kt in range(ts_s):
                                ktg = t_off + kt
                                if all_unit:
                                    p_t = pT_all[:, qi, :, :, :]
                                else:
                                    p_t = pp.tile([P, 2, 4, P], BF16,
                                                  tag="pT", name="p_t")
                                stp = psA.tile([P, 2, 4, P], F32, tag="work2",
                                               name="stp")
                                for b in range(2):
                                    for hh in range(4):
                                        nc.tensor.matmul(
                                            stp[:, b, hh, :],
                                            khead(hh, b, slice(
                                                ktg * P, (ktg + 1) * P)),
                                            qhead(hh, b, slice(
                                                qc0, qc0 + P)),
                                            start=True,
                                            stop=True,
                                            tile_position=stile(hh),
                                        )
                                # one exp over both head-banks (2-bank read)
                                nc.scalar.activation(
                                    p_t,
                                    stp,
                                    EXP,
                                    scale=SCALE,
                                    bias=ccol(C_FIXED + ktg),
                                )
                                pT[kt] = p_t
                            # PV accumulation, one closed psum group per
                            # (bank, head) at a time
                            for b in range(2):
                                for hh in range(4):
                                    for kt in range(ts_s):
                                        nc.tensor.matmul(
                                            oT_ps[32 * hh : 32 * hh + 32,
                                                  b, qrel : qrel + P],
                                            v_sb[:, t_off + kt,
                                                 (4 * b + hh) * 32 : (4 * b + hh) * 32 + 32],
                                            pT[kt][:, b, hh, :],
                                            start=(kt == 0),
                                            stop=(kt == ts_s - 1),
                                            tile_position=(0, 32 * hh),
                                        )
                                    if not all_unit:
                                        for kt in range(ts_s):
                                            nc.tensor.matmul(
                                                dn_ps[32 * hh : 32 * hh + 32,
                                                      b, qrel : qrel + P],
                                                ones32,
                                                pT[kt][:, b, hh, :],
                                                start=(kt == 0),
                                                stop=(kt == ts_s - 1),
                                                tile_position=(0, 32 * hh),
                                            )
                        t_off += ts_s
                        q_off += ts_s * P
                    if all_unit:
                        # merged denominators: each slot's keys live on the
                        # partition axis of its own pT block
                        dn_ps = psA.tile([P, 2, cw], F32, tag="work2",
                                         name="dn")
                        for b in range(2):
                            for hh in range(4):
                                nc.tensor.matmul(
                                    dn_ps[32 * hh : 32 * hh + 32, b, :],
                                    ones32,
                                    pT_all[:, :, b, hh, :],
                                    start=True,
                                    stop=True,
                                    tile_position=(0, 32 * hh),
                                )
                    # normalize: o = oT * (1/dn), fused with PSUM->SBUF copy
                    r_sb = pp.tile([P, 2, cw], F32, tag="r", name="r_sb")
                    nc.vector.reciprocal_approx_fast(out=r_sb, in_=dn_ps)
                    nc.vector.tensor_mul(o_sb[:, :, c0 : c0 + cw], oT_ps, r_sb)
                # output projection + residual
                y = xp.tile([P, 2, NC], dt, tag=f"y{i}", name=f"y{i}")
                for c0, cw in NCH:
                    zp = psA.tile([P, 2, cw], F32, tag="work", name="zp")
                    for j in range(2):
                        for k in range(2):
                            nc.tensor.matmul(
                                zp[:, j, :],
                                r32(wcol(w_base(i, "o"), k, j)),
                                r32(o_sb[:, k, c0 : c0 + cw]),
                                start=(k == 0),
                                stop=(k == 1),
                            )
                    if trivial_b:
                        nc.vector.tensor_add(
                            y[:, :, c0 : c0 + cw], zp, hp[:, :, c0 : c0 + cw])
                    else:
                        for j in range(2):
                            nc.vector.scalar_tensor_tensor(
                                y[:, j, c0 : c0 + cw], zp[:, j, :],
                                ccol(c_blk(i, "o", j)),
                                hp[:, j, c0 : c0 + cw],
                                op0=mybir.AluOpType.add,
                                op1=mybir.AluOpType.add,
                            )
                out_dt = F32 if i == 1 else dt
                # mean(y) = rowmean(Wo).T @ o + mean(h): computable while the
                # O-projection itself still runs (only valid with zero bo)
                mean_terms = None
                if trivial_b:
                    mean_terms = [
                        (lambda k, i=i: wormcol(i, k), o_sb),
                        (lambda k: c256rep[:], hp),
                    ]
                return layernorm_pair(
                    y, c_blk(i, "g", 0), c_blk(i, "beta", 0), out_dt,
                    tag=f"x{i + 1}", mean_terms=mean_terms,
                )

            # block 0 pre-layer: LN(tanh(x @ lin_W + lin_b)) * g + beta,
            # with Tanh straight from ACT table set 0.
            t0 = xp.tile([P, 2, NC], dt, tag="t0", name="t0")
            for c0, cw in NCH:
                ps = psA.tile([P, 2, cw], F32, tag="work", name="lin_ps")
                for j in range(2):
                    for k in range(2):
                        nc.tensor.matmul(
                            ps[:, j, :],
                            r32(wcol(LIN_BASE, k, j)),
                            r32(x0[:, k, c0 : c0 + cw]),
                            start=(k == 0),
                            stop=(k == 1),
                        )
                if trivial_b:
                    nc.scalar.activation(t0[:, :, c0 : c0 + cw], ps, TANH)
                else:
                    for j in range(2):
                        nc.scalar.activation(
                            t0[:, j, c0 : c0 + cw], ps[:, j, :], TANH,
                            bias=ccol(c_lin("b", j)))
            h0 = layernorm_pair(t0, c_lin("g", 0), c_lin("beta", 0), dt,
                                tag="h0")
            x1 = attention_block(0, h0)
            x2 = attention_block(1, x1)
            nc.sync.dma_start(outT[:, :, 0:h_nc], x2[:, :, 0:h_nc])
            nc.scalar.dma_start(outT[:, :, h_nc:NC], x2[:, :, h_nc:NC])

    nc.finalize()
    return nc


# ---------------------------------------------------------------------------
# host side
# ---------------------------------------------------------------------------
_prog_cache = {}
_last_results = None


def _get_program(slot_ts, trivial_ln, trivial_b):
    key = (tuple(slot_ts), trivial_ln, trivial_b, DT_ACT_NAME)
    if key not in _prog_cache:
        _prog_cache[key] = build_program(tuple(slot_ts), trivial_ln, trivial_b)
    return _prog_cache[key]


def _segments(ordering):
    """contiguous runs of equal values in sorted ordering -> (start, len)."""
    n = ordering.shape[0]
    change = np.nonzero(np.diff(ordering))[0] + 1
    starts = np.concatenate([[0], change])
    lens = np.diff(np.concatenate([starts, [n]]))
    return list(zip(starts.tolist(), lens.tolist()))


def kernel(
    feat, ordering, lin_W, lin_b, lin_g, lin_beta,
    Wq, Wk, Wv, bq, bk, bv, Wo, bo, ln_g, ln_b,
):
    feat = np.asarray(feat, np.float32)
    ordering = np.asarray(ordering)
    N = feat.shape[0]
    np_act = _np_act()

    perm = np.argsort(ordering, kind="stable")
    segs = _segments(np.asarray(ordering)[perm])

    # deal segments (sorted by length desc) snake-wise to cores
    order = sorted(range(len(segs)), key=lambda i: -segs[i][1])
    core_slots = [[] for _ in range(N_CORES)]
    for r, si in enumerate(order):
        c = r % (2 * N_CORES)
        c = c if c < N_CORES else 2 * N_CORES - 1 - c
        core_slots[c].append(si)
    S = max(len(cs) for cs in core_slots)
    # per-slot tile count = max over cores of the slot's segment size
    slot_ts = []
    for k in range(S):
        mx = 1
        for c in range(N_CORES):
            if k < len(core_slots[c]):
                mx = max(mx, (segs[core_slots[c][k]][1] + P - 1) // P)
        slot_ts.append(mx)
    T = sum(slot_ts)
    NC = T * P

    trivial_ln = bool(
        np.all(np.asarray(ln_g) == 1) and np.all(np.asarray(ln_b) == 0)
        and np.all(np.asarray(lin_g) == 1) and np.all(np.asarray(lin_beta) == 0)
    )
    # softmax rows sum to 1, so attn @ (v + 1*bv^T) = attn@v + 1*bv^T; the
    # per-block value bias folds exactly into the output-projection bias:
    # bo_eff = bo + bv @ Wo
    bo = np.asarray(bo, np.float32) + np.einsum(
        "id,idj->ij", np.asarray(bv, np.float32), np.asarray(Wo, np.float32)
    )

    trivial_b = bool(
        np.all(np.asarray(lin_b) == 0) and np.all(np.asarray(bq) == 0)
        and np.all(np.asarray(bk) == 0) and np.all(np.asarray(bo) == 0)
    )
    nc = _get_program(slot_ts, trivial_ln, trivial_b)

    # ---- pack weights ----
    wallp = np.zeros((P, N_WCOLS), np.float32)

    def put_w(base, W):
        W = np.asarray(W, np.float32)
        for k in range(2):
            for j in range(2):
                c0 = base + k * 256 + j * 128
                wallp[:, c0 : c0 + 128] = W[k * 128 : (k + 1) * 128,
                                            j * 128 : (j + 1) * 128]

    put_w(LIN_BASE, lin_W)
    for i in range(2):
        put_w(w_base(i, "q"), np.asarray(Wq)[i])
        put_w(w_base(i, "k"), np.asarray(Wk)[i])
        put_w(w_base(i, "v"), np.asarray(Wv)[i])
        put_w(w_base(i, "o"), np.asarray(Wo)[i])
        # replicated row-mean tiles of Wo for the residual-mean matmuls:
        # mean_d((Wo.T o)_d) = sum_e rowmean(Wo)[e] * o[e]
        wo_rm = np.asarray(Wo)[i].astype(np.float32).mean(axis=1)
        for k in range(2):
            c0 = WO_RM + i * 256 + k * 128
            wallp[:, c0 : c0 + 128] = np.repeat(
                wo_rm[k * 128 : (k + 1) * 128][:, None], 128, axis=1)
    wallp = wallp.astype(np_act)

    consp = np.zeros((P, C_FIXED + T), np.float32)

    def put_c(idx, vec):
        vec = np.asarray(vec, np.float32)
        consp[:, idx] = vec[:128]
        consp[:, idx + 1] = vec[128:]

    put_c(c_lin("b", 0), np.asarray(lin_b, np.float32))
    put_c(c_lin("g", 0), lin_g)
    put_c(c_lin("beta", 0), lin_beta)
    for i in range(2):
        put_c(c_blk(i, "q", 0), np.asarray(bq)[i])
        put_c(c_blk(i, "k", 0), np.asarray(bk)[i])
        put_c(c_blk(i, "v", 0), np.asarray(bv)[i] * 0)
        put_c(c_blk(i, "o", 0), np.asarray(bo)[i])
        put_c(c_blk(i, "g", 0), np.asarray(ln_g)[i])
        put_c(c_blk(i, "beta", 0), np.asarray(ln_b)[i])

    # ---- per-core data ----
    feat_sorted = feat[perm]
    in_maps = []
    core_meta = []  # (slot k, seg_start, seg_len, node_offset in padded layout)
    for c in range(N_CORES):
        fT = np.zeros((NC, 256), np.float32)
        mb = np.full((NC,), NEG, np.float32)
        meta = []
        off = 0
        for k in range(S):
            if k < len(core_slots[c]):
                st, ln = segs[core_slots[c][k]]
                fT[off : off + ln] = feat_sorted[st : st + ln]
                mb[off : off + ln] = 0.0
                meta.append((st, ln, off))
            off += slot_ts[k] * P
        cons_c = consp.copy()
        cons_c[:, C_FIXED : C_FIXED + T] = mb.reshape(T, P).T
        featT_c = np.ascontiguousarray(
            fT.T.reshape(2, P, NC).transpose(1, 0, 2)
        ).astype(np_act)
        in_maps.append({"featT": featT_c, "wall": wallp, "cons": cons_c})
        core_meta.append(meta)

    res = run_bass_kernel_spmd(nc, in_maps, list(range(N_CORES)))
    global _last_results
    _last_results = res

    out = np.empty((N, 256), np.float32)
    for c in range(N_CORES):
        oT = np.asarray(res.results[c]["outT"], np.float32)  # [128, 2, NC]
        o_nodes = oT.transpose(1, 0, 2).reshape(256, NC).T  # [NC, 256]
        for st, ln, off in core_meta[c]:
            out[perm[st : st + ln]] = o_nodes[off : off + ln]
    return out


# revision 13
# speedup vs baseline: 1.2183x; 1.0248x over previous
"""Trainium2 Bass kernel for BatchedStarNetAttentionBlock.

Strategy: data-parallel over ordering segments (attention is block-diagonal,
never crosses segment boundaries). Each of the 8 cores gets a subset of
segments, padded to a shared static structure so one SPMD program serves all
cores. No collectives.

On-device layout: activations are kept transposed, xT[d, n] with the feature
dim on partitions (2 tiles of 128), so every linear layer is a natural
matmul (lhsT = weight chunk [k,j], rhs = xT chunk [k,n]). Scores are computed
directly in transposed form S.T = kT.T @ qT with the per-head 32-row slices
of kT/qT addressed in place at partition offsets 32h via PE row-group
tile_position (no SBUF shuffle, 4 heads run concurrently in the PE array).
PV is col-tiled by head so attention output lands as oT[d, n] in PSUM.
Denominators come from ones-matmuls writing a row-replicated bank with the
same layout as oT.

LayerNorm uses replicated-mean matmuls (lhsT = [128,128] tile of 1/256) so
the mean arrives in PSUM already broadcast across partitions: no row ops and
no separate broadcast matmuls. var = E[(y-m)^2] via a second replicated
matmul over z^2; rstd = exp(-0.5*ln(var+eps)) on full-width tiles. The mean
of a post-attention residual y = Wo.T o + h is computed before y itself
exists, via lhsT tiles of row-means of Wo (host-packed) plus 1/256 against
h, overlapping the O-projection.

Block0's Tanh uses the ACT Tanh entry directly (activation-table planner is
doctored so table set 0 claims only Tanh; everything else lives in the
natural_log_exp set -> exactly two table loads, both off critical path).

Extra touches: dummy warm-up matmuls keep the PE HAM clock-gate from running
the first real matmuls at 1.2 GHz; input DMAs are split across the three
DMA-capable queues in consumption order; elementwise work is spread across
DVE / ACT / GpSimd.
"""

import sys

for _p in ("/opt/trn_rl_repo",):
    if _p not in sys.path:
        sys.path.insert(0, _p)

import numpy as np
import ml_dtypes

import bass_rust as _bass_rust

import concourse.bass as bass
import concourse.tile as tile
from concourse import bacc
from concourse import mybir
from concourse.bass_utils import run_bass_kernel_spmd
from concourse.hw_specs import get_activation_tables


class _Bacc(bacc.Bacc):
    """Bacc whose activation-table planner maps Tanh to table set 0 and every
    other used function (Ln/Exp/Square/Identity/Copy) to the one set that
    contains them all, so the program pays exactly two ACT table loads."""

    def insert_act_table_loads(self):
        has_activation = any(
            isinstance(i, mybir.InstActivation)
            for b in self.main_func.blocks
            for i in b.instructions
        )
        if not has_activation:
            return
        tables = list(get_activation_tables(self.m.arch).items())
        # The planner emits act_func_set_id = position in this list, so
        # positions must stay aligned with act_info.json. Narrow the match
        # sets instead: position 0 claims only tanh; other sets before
        # natural_log_exp_and_others claim nothing; so exp/ln/square/
        # identity/copy all resolve to the one set that has them all.
        pref = "natural_log_exp_and_others"
        TANH = mybir.ActivationFunctionType.Tanh
        doctored = []
        seen_pref = False
        for name, fns in tables:
            if name == pref:
                seen_pref = True
                doctored.append((name, fns))
            elif not seen_pref:
                doctored.append((name, {TANH} & fns))
            else:
                doctored.append((name, fns))
        _bass_rust.insert_act_table_loads(self, doctored)

P = 128
D = 256
H = 8
DH = 32
SCALE = 1.0 / float(np.sqrt(DH))
N_CORES = 8
NEG = -1e9

F32 = mybir.dt.float32
BF16 = mybir.dt.bfloat16

# activation dtype switch ("f32" or "bf16")
DT_ACT_NAME = "bf16"

# score matmuls read qT/kT head slices in place at partition offsets 32h
# (PE row groups); False falls back to SBUF->SBUF shuffles to partition 0
DIRECT_SCORES = True


def _dt_act():
    return BF16 if DT_ACT_NAME == "bf16" else F32


def _np_act():
    return ml_dtypes.bfloat16 if DT_ACT_NAME == "bf16" else np.float32


# ---------------------------------------------------------------------------
# weight packing layout (shared between host packer and device program)
# ---------------------------------------------------------------------------
# W_all [128, n_wcols] (dt_act): matmul weight chunks, 128 cols each.
#   chunk_col(base, k, j) = base + k*(2*128) + j*128   (k-outer, j-inner)
#   lin_W  at base 0                      (4 chunks)
#   Wq[i]  at 512 + i*2048 + 0
#   Wk[i]  at 512 + i*2048 + 512
#   Wv[i]  at 512 + i*2048 + 1024
#   Wo[i]  at 512 + i*2048 + 1536
#   wo_rm[i][k] at 4608 + i*256 + k*128 : row-means of Wo[i] (over the output
#     dim), replicated across the 128 columns -> replicated-mean lhsT tiles.
N_WCOLS = 512 + 2 * 2048 + 512

LIN_BASE = 0
WO_RM = 512 + 2 * 2048


def w_base(i, which):
    return 512 + i * 2048 + {"q": 0, "k": 512, "v": 1024, "o": 1536}[which]


# C_all [128, n_ccols] f32: per-feature columns (partition = feature within
# d-tile j). col index helpers:
#   0,1   lin_b (j=0,1)
#   2,3   lin_g
#   4,5   lin_beta
#   6+i*12 + [0,1]=bq, [2,3]=bk, [4,5]=bv, [6,7]=bo, [8,9]=ln_g, [10,11]=ln_b
#   30..30+T  maskbias columns (per key-tile)
def c_lin(which, j):
    return {"b": 0, "g": 2, "beta": 4}[which] + j


def c_blk(i, which, j):
    return 6 + i * 12 + {"q": 0, "k": 2, "v": 4, "o": 6, "g": 8, "beta": 10}[which] + j


C_FIXED = 30


# ---------------------------------------------------------------------------
# device program
# ---------------------------------------------------------------------------
def build_program(slot_ts, trivial_ln, trivial_b):
    """slot_ts: tuple of per-slot tile counts (shared across cores).
    trivial_ln: all LN gains are 1 and shifts 0.
    trivial_b: all linear-layer biases are zero (enables pair-merged ops).
    """
    dt = _dt_act()
    T = int(sum(slot_ts))
    NC = T * P  # padded node count per core
    CHW = 256  # chunk width for the n dimension
    NCH = [(c0, min(CHW, NC - c0)) for c0 in range(0, NC, CHW)]  # n chunks

    TANH = mybir.ActivationFunctionType.Tanh
    LN_F = mybir.ActivationFunctionType.Ln
    EXP = mybir.ActivationFunctionType.Exp
    IDENT = mybir.ActivationFunctionType.Identity

    nc = _Bacc()
    featT = nc.declare_dram_parameter("featT", [P, 2, NC], dt, isOutput=False)
    wall = nc.declare_dram_parameter("wall", [P, N_WCOLS], dt, isOutput=False)
    cons = nc.declare_dram_parameter("cons", [P, C_FIXED + T], F32, isOutput=False)
    outT = nc.declare_dram_parameter("outT", [P, 2, NC], F32, isOutput=True)

    with tile.TileContext(nc) as tc:
        with (
            tc.tile_pool(name="wp", bufs=1) as wp,
            tc.tile_pool(name="xp", bufs=1) as xp,
            tc.tile_pool(name="pp", bufs=2) as pp,
            tc.tile_pool(name="psA", bufs=2, space="PSUM") as psA,
            tc.tile_pool(name="psB", bufs=2, space="PSUM") as psB,
            tc.tile_pool(name="psS", bufs=1, space="PSUM") as psS,
        ):
            # PSUM: psA tag "work" = 1-bank tiles (x2); psB tag "oT" 1-bank
            # (x2); psS tag "stp4" = 4-bank score tile (x1: each of the 4
            # concurrent PE row groups must own a whole PSUM bank). Total 8.
            w_lin = wp.tile([P, 512], dt, tag="w_lin")
            w_blk = [wp.tile([P, 2048], dt, tag=f"w_blk{i}", name=f"w_blk{i}")
                     for i in range(2)]
            worm = wp.tile([P, 512], dt, tag="worm")
            c_sb = wp.tile([P, C_FIXED + T], F32, tag="c")
            x0 = xp.tile([P, 2, NC], dt, tag="x0", name="x0")

            # input DMAs, split across the three DMA-capable queues in
            # consumption order: lin weights + x0 first, then block0's
            # attention weights, then block1's.
            h_nc = NC // 2
            nc.sync.dma_start(x0[:, :, 0:h_nc], featT[:, :, 0:h_nc])
            nc.scalar.dma_start(w_lin[:], wall[:, 0:512])
            nc.sync.dma_start(x0[:, :, h_nc:NC], featT[:, :, h_nc:NC])
            nc.gpsimd.dma_start(c_sb[:], cons[:])
            nc.gpsimd.dma_start(worm[:], wall[:, WO_RM : WO_RM + 512])
            nc.scalar.dma_start(w_blk[0][:, 0:1024], wall[:, 512:1536])
            nc.gpsimd.dma_start(w_blk[0][:, 1024:2048], wall[:, 1536:2560])
            nc.sync.dma_start(w_blk[1][:], wall[:, 2560:4608])

            def w_tile_of(base):
                if base < 512:
                    return w_lin, base
                if base >= WO_RM:
                    return worm, base - WO_RM
                i = (base - 512) // 2048
                return w_blk[i], (base - 512) % 2048

            # constants
            ones32 = wp.tile([P, 32], BF16, tag="ones32")
            nc.vector.memset(ones32, 1.0)
            c256rep = wp.tile([P, P], BF16, tag="c256rep")
            nc.vector.memset(c256rep, 1.0 / 256.0)
            eps_col = wp.tile([P, 1], F32, tag="eps_col")
            nc.vector.memset(eps_col, 1e-5)

            def repf(ap, n):
                # repeat a [P, w] AP along a stride-0 middle dim -> [P, n, w]
                return bass.AP(tensor=ap.tensor, offset=ap.offset,
                               ap=[list(ap.ap[0]), [0, n]] + [list(a) for a in ap.ap[1:]])

            # HAM warm-up: keep the PE busy for ~4us while inputs stream in,
            # so the clock gate is released before the first real matmul.
            hamrhs = wp.tile([P, 512], BF16, tag="hamrhs")
            nc.vector.memset(hamrhs, 0.5)
            ham = psA.tile([P, 512], F32, tag="work", name="ham")
            for _ in range(8):
                nc.tensor.matmul(ham[:], c256rep[:], hamrhs[:],
                                 start=True, stop=True)

            def wcol(base, k, j, width=P):
                wt, rel = w_tile_of(base)
                c0 = rel + k * 256 + j * 128
                return wt[:, c0 : c0 + width]

            def wormcol(i, k):
                return worm[:, i * 256 + k * 128 : i * 256 + k * 128 + P]

            def ccol(idx):
                return c_sb[:, idx : idx + 1]

            def r32(ap):
                # float32r: same bits as f32, single-pass PE mode (vs the
                # 2-pass LOW_HIGH fp32 lowering)
                return ap.bitcast(mybir.dt.float32r) if ap.dtype == F32 else ap

            def rep2(ap):
                # repeat a [P, w] AP along a stride-0 middle dim -> [P, 2, w]
                return repf(ap, 2)

            def proj_pair(src, base, bias_idx, out_dt, func=None, scale=1.0,
                          eng="act", tag=None):
                """pair projection: out[:, j, n] = func(W_j.T @ src + b_j)."""
                out = xp.tile([P, 2, NC], out_dt, tag=tag or f"pj{base}",
                              name=tag or f"pj{base}")
                func = func or IDENT
                for c0, cw in NCH:
                    ps = psA.tile([P, 2, cw], F32, tag="work", name="pp2")
                    for j in range(2):
                        for k in range(2):
                            nc.tensor.matmul(
                                ps[:, j, :],
                                r32(wcol(base, k, j)),
                                r32(src[:, k, c0 : c0 + cw]),
                                start=(k == 0),
                                stop=(k == 1),
                            )
                    dst = out[:, :, c0 : c0 + cw]
                    if trivial_b:
                        if eng == "dve":
                            nc.vector.tensor_copy(dst, ps)
                        else:
                            nc.scalar.activation(dst, ps, func, scale=scale)
                    else:
                        for j in range(2):
                            if eng == "dve":
                                nc.vector.tensor_scalar_add(
                                    out[:, j, c0 : c0 + cw], ps[:, j, :],
                                    ccol(bias_idx + j))
                            else:
                                nc.scalar.activation(
                                    out[:, j, c0 : c0 + cw], ps[:, j, :], func,
                                    bias=ccol(bias_idx + j), scale=scale)
                return out

            def layernorm_pair(y, gcol, bcol, out_dt, tag, mean_terms=None):
                """LayerNorm over the feature dim (partition axis, 2 tiles).

                mean_terms: optional list of (lhsT_fn(k) -> AP, src_pair)
                whose accumulated matmuls give the replicated mean of y;
                defaults to [(1/256 tile, y)]. Lets callers overlap the mean
                with producing y itself.
                """
                out = xp.tile([P, 2, NC], out_dt, tag=tag, name=tag)
                z = xp.tile([P, 2, NC], dt, tag=f"{tag}_z", name=f"{tag}_z")
                sq = xp.tile([P, 2, NC], dt, tag=f"{tag}_sq", name=f"{tag}_sq")
                if mean_terms is None:
                    mean_terms = [(lambda k: c256rep[:], y)]
                for c0, cw in NCH:
                    m_b = psA.tile([P, cw], F32, tag="work", name="m_b")
                    n_mm = 2 * len(mean_terms)
                    idx = 0
                    for lf, src in mean_terms:
                        for k in range(2):
                            nc.tensor.matmul(
                                m_b[:], r32(lf(k)),
                                r32(src[:, k, c0 : c0 + cw]),
                                start=(idx == 0), stop=(idx == n_mm - 1),
                            )
                            idx += 1
                    zc = z[:, :, c0 : c0 + cw]
                    nc.vector.tensor_sub(zc, y[:, :, c0 : c0 + cw], rep2(m_b))
                    sqc = sq[:, :, c0 : c0 + cw]
                    nc.gpsimd.tensor_mul(sqc, zc, zc)
                    var_b = psA.tile([P, cw], F32, tag="work", name="var_b")
                    for k in range(2):
                        nc.tensor.matmul(
                            var_b[:], r32(c256rep[:]),
                            r32(sq[:, k, c0 : c0 + cw]),
                            start=(k == 0), stop=(k == 1),
                        )
                    # rstd = exp(-0.5*ln(var+eps)) on the replicated tile
                    lnv = pp.tile([P, cw], F32, tag="lnv", name="lnv")
                    nc.scalar.activation(lnv, var_b, LN_F, bias=eps_col[:])
                    rstd = pp.tile([P, cw], dt, tag="rstd", name="rstd")
                    nc.scalar.activation(rstd, lnv, EXP, scale=-0.5)
                    dst = out[:, :, c0 : c0 + cw]
                    if trivial_ln:
                        nc.vector.tensor_mul(dst, zc, rep2(rstd))
                    else:
                        t2 = xp.tile([P, 2, cw], dt, tag="ln_t2", name="ln_t2")
                        nc.vector.tensor_mul(t2, zc, rep2(rstd))
                        for k in range(2):
                            nc.vector.tensor_scalar(
                                out[:, k, c0 : c0 + cw], t2[:, k, :],
                                ccol(gcol + k), ccol(bcol + k),
                                op0=mybir.AluOpType.mult,
                                op1=mybir.AluOpType.add,
                            )
                return out

            def attention_block(i, hp):
                """one MHA block on the pair tile hp; returns new x pair."""
                qT = proj_pair(hp, w_base(i, "q"), c_blk(i, "q", 0), BF16,
                               tag=f"qT{i}")
                kT = proj_pair(hp, w_base(i, "k"), c_blk(i, "k", 0), BF16,
                               eng="dve", tag=f"kT{i}")
                # v in node layout: v[n_tile, d] = hT_chunk.T @ Wv_chunk
                v_sb = xp.tile([P, T, 256], BF16, tag=f"v{i}", name=f"v{i}")
                for t2i in range(0, T, 2):
                    npair = min(2, T - t2i)
                    vp = psA.tile([P, npair, 256], F32, tag="work", name="vp")
                    for tt in range(npair):
                        t = t2i + tt
                        for k in range(2):
                            nc.tensor.matmul(
                                vp[:, tt, :],
                                r32(hp[:, k, t * P : (t + 1) * P]),
                                r32(wcol(w_base(i, "v"), k, 0, width=256)),
                                start=(k == 0),
                                stop=(k == 1),
                            )
                    nc.vector.tensor_copy(v_sb[:, t2i : t2i + npair, :], vp)
                if not DIRECT_SCORES:
                    q_h = [xp.tile([32, NC], BF16, tag=f"q{i}h{h}",
                                   name=f"q{i}h{h}") for h in range(H)]
                    k_h = [xp.tile([32, NC], BF16, tag=f"k{i}h{h}",
                                   name=f"k{i}h{h}") for h in range(H)]
                    engs = [nc.sync, nc.gpsimd, nc.scalar]
                    for h in range(H):
                        b, hh = divmod(h, 4)
                        engs[h % 3].dma_start(
                            q_h[h][:], qT[32 * hh : 32 * hh + 32, b, :])
                        engs[(h + 1) % 3].dma_start(
                            k_h[h][:], kT[32 * hh : 32 * hh + 32, b, :])

                    def khead(hh, b, cslice):
                        return k_h[4 * b + hh][:, cslice]

                    def qhead(hh, b, cslice):
                        return q_h[4 * b + hh][:, cslice]
                # attention per slot; scores read qT/kT per-head slices in
                # place at partition offset 32h (PE row groups, 4 heads
                # concurrent in the array)
                o_sb = xp.tile([P, 2, NC], dt, tag=f"o{i}", name=f"o{i}")
                all_unit = all(t == 1 for t in slot_ts)
                for c0, cw in NCH:
                    oT_ps = psB.tile([P, 2, cw], F32, tag="oT", name="oT")
                    dn_ps = None
                    if not all_unit:
                        dn_ps = psA.tile([P, 2, cw], F32, tag="work",
                                         name="dn")
                    ctiles = cw // P
                    pT_all = pp.tile([P, ctiles, 2, 4, P], BF16, tag="pTall",
                                     name="pT_all")
                    t_off = 0
                    q_off = 0
                    for s, ts_s in enumerate(slot_ts):
                        for qc0 in range(q_off, q_off + ts_s * P, P):
                            if qc0 < c0 or qc0 >= c0 + cw:
                                continue
                            qrel = qc0 - c0
                            qi = qrel // P
                            pT = {}
                            for kt in range(ts_s):
                                ktg = t_off + kt
                                if all_unit:
                                    p_t = pT_all[:, qi, :, :, :]
                                else:
                                    p_t = pp.tile([P, 2, 4, P], BF16,
                                                  tag="pT", name="p_t")
                                if DIRECT_SCORES:
                                    # 4-bank score tile: PE row group hh owns
                                    # bank hh (concurrent row tiles must not
                                    # share a PSUM bank)
                                    stp4 = psS.tile([P, 4, 512], F32,
                                                    tag="stp4", name="stp4")
                                    for b in range(2):
                                        for hh in range(4):
                                            nc.tensor.matmul(
                                                stp4[:, hh, b * P : (b + 1) * P],
                                                kT[32 * hh : 32 * hh + 32, b,
                                                   ktg * P : (ktg + 1) * P],
                                                qT[32 * hh : 32 * hh + 32, b,
                                                   qc0 : qc0 + P],
                                                start=True,
                                                stop=True,
                                                tile_position=(32 * hh, 0),
                                            )
                                    # exp reads (b, hh, col)-strided view so
                                    # pT keeps its (bank, head) layout
                                    s4 = stp4[:]
                                    src = bass.AP(
                                        tensor=s4.tensor, offset=s4.offset,
                                        ap=[list(s4.ap[0]), [P, 2], [512, 4],
                                            [1, P]])
                                    nc.scalar.activation(
                                        p_t, src, EXP,
                                        scale=SCALE, bias=ccol(C_FIXED + ktg))
                                else:
                                    stp = psA.tile([P, 2, 4, P], F32,
                                                   tag="work2", name="stp")
                                    for b in range(2):
                                        for hh in range(4):
                                            nc.tensor.matmul(
                                                stp[:, b, hh, :],
                                                khead(hh, b, slice(
                                                    ktg * P, (ktg + 1) * P)),
                                                qhead(hh, b, slice(
                                                    qc0, qc0 + P)),
                                                start=True,
                                                stop=True,
                                            )
                                    # one exp over both head-banks
                                    nc.scalar.activation(
                                        p_t, stp, EXP,
                                        scale=SCALE, bias=ccol(C_FIXED + ktg))
                                pT[kt] = p_t
                            # PV accumulation, one closed psum group per
                            # (bank, head) at a time
                            for b in range(2):
                                for hh in range(4):
                                    for kt in range(ts_s):
                                        nc.tensor.matmul(
                                            oT_ps[32 * hh : 32 * hh + 32,
                                                  b, qrel : qrel + P],
                                            v_sb[:, t_off + kt,
                                                 (4 * b + hh) * 32 : (4 * b + hh) * 32 + 32],
                                            pT[kt][:, b, hh, :],
                                            start=(kt == 0),
                                            stop=(kt == ts_s - 1),
                                            tile_position=(0, 32 * hh),
                                        )
                                    if not all_unit:
                                        for kt in range(ts_s):
                                            nc.tensor.matmul(
                                                dn_ps[32 * hh : 32 * hh + 32,
                                                      b, qrel : qrel + P],
                                                ones32,
                                                pT[kt][:, b, hh, :],
                                                start=(kt == 0),
                                                stop=(kt == ts_s - 1),
                                                tile_position=(0, 32 * hh),
                                            )
                        t_off += ts_s
                        q_off += ts_s * P
                    if all_unit:
                        # merged denominators: each slot's keys live on the
                        # partition axis of its own pT block
                        dn_ps = psA.tile([P, 2, cw], F32, tag="work",
                                         name="dn")
                        for b in range(2):
                            for hh in range(4):
                                nc.tensor.matmul(
                                    dn_ps[32 * hh : 32 * hh + 32, b, :],
                                    ones32,
                                    pT_all[:, :, b, hh, :],
                                    start=True,
                                    stop=True,
                                    tile_position=(0, 32 * hh),
                                )
                    # normalize: o = oT * (1/dn), fused with PSUM->SBUF copy
                    r_sb = pp.tile([P, 2, cw], F32, tag="r", name="r_sb")
                    nc.vector.reciprocal_approx_fast(out=r_sb, in_=dn_ps)
                    nc.vector.tensor_mul(o_sb[:, :, c0 : c0 + cw], oT_ps, r_sb)
                # output projection + residual
                y = xp.tile([P, 2, NC], dt, tag=f"y{i}", name=f"y{i}")
                for c0, cw in NCH:
                    zp = psA.tile([P, 2, cw], F32, tag="work", name="zp")
                    for j in range(2):
                        for k in range(2):
                            nc.tensor.matmul(
                                zp[:, j, :],
                                r32(wcol(w_base(i, "o"), k, j)),
                                r32(o_sb[:, k, c0 : c0 + cw]),
                                start=(k == 0),
                                stop=(k == 1),
                            )
                    if trivial_b:
                        nc.vector.tensor_add(
                            y[:, :, c0 : c0 + cw], zp, hp[:, :, c0 : c0 + cw])
                    else:
                        for j in range(2):
                            nc.vector.scalar_tensor_tensor(
                                y[:, j, c0 : c0 + cw], zp[:, j, :],
                                ccol(c_blk(i, "o", j)),
                                hp[:, j, c0 : c0 + cw],
                                op0=mybir.AluOpType.add,
                                op1=mybir.AluOpType.add,
                            )
                out_dt = F32 if i == 1 else dt
                # mean(y) = rowmean(Wo).T @ o + mean(h): computable while the
                # O-projection itself still runs (only valid with zero bo)
                mean_terms = None
                if trivial_b:
                    mean_terms = [
                        (lambda k, i=i: wormcol(i, k), o_sb),
                        (lambda k: c256rep[:], hp),
                    ]
                return layernorm_pair(
                    y, c_blk(i, "g", 0), c_blk(i, "beta", 0), out_dt,
                    tag=f"x{i + 1}", mean_terms=mean_terms,
                )

            # block 0 pre-layer: LN(tanh(x @ lin_W + lin_b)) * g + beta,
            # with Tanh straight from ACT table set 0.
            t0 = xp.tile([P, 2, NC], dt, tag="t0", name="t0")
            for c0, cw in NCH:
                ps = psA.tile([P, 2, cw], F32, tag="work", name="lin_ps")
                for j in range(2):
                    for k in range(2):
                        nc.tensor.matmul(
                            ps[:, j, :],
                            r32(wcol(LIN_BASE, k, j)),
                            r32(x0[:, k, c0 : c0 + cw]),
                            start=(k == 0),
                            stop=(k == 1),
                        )
                if trivial_b:
                    nc.scalar.activation(t0[:, :, c0 : c0 + cw], ps, TANH)
                else:
                    for j in range(2):
                        nc.scalar.activation(
                            t0[:, j, c0 : c0 + cw], ps[:, j, :], TANH,
                            bias=ccol(c_lin("b", j)))
            h0 = layernorm_pair(t0, c_lin("g", 0), c_lin("beta", 0), dt,
                                tag="h0")
            x1 = attention_block(0, h0)
            x2 = attention_block(1, x1)
            nc.sync.dma_start(outT[:, :, 0:h_nc], x2[:, :, 0:h_nc])
            nc.scalar.dma_start(outT[:, :, h_nc:NC], x2[:, :, h_nc:NC])

    nc.finalize()
    return nc


# ---------------------------------------------------------------------------
# host side
# ---------------------------------------------------------------------------
_prog_cache = {}
_last_results = None


def _get_program(slot_ts, trivial_ln, trivial_b):
    key = (tuple(slot_ts), trivial_ln, trivial_b, DT_ACT_NAME)
    if key not in _prog_cache:
        _prog_cache[key] = build_program(tuple(slot_ts), trivial_ln, trivial_b)
    return _prog_cache[key]


def _segments(ordering):
    """contiguous runs of equal values in sorted ordering -> (start, len)."""
    n = ordering.shape[0]
    change = np.nonzero(np.diff(ordering))[0] + 1
    starts = np.concatenate([[0], change])
    lens = np.diff(np.concatenate([starts, [n]]))
    return list(zip(starts.tolist(), lens.tolist()))


def kernel(
    feat, ordering, lin_W, lin_b, lin_g, lin_beta,
    Wq, Wk, Wv, bq, bk, bv, Wo, bo, ln_g, ln_b,
):
    feat = np.asarray(feat, np.float32)
    ordering = np.asarray(ordering)
    N = feat.shape[0]
    np_act = _np_act()

    perm = np.argsort(ordering, kind="stable")
    segs = _segments(np.asarray(ordering)[perm])

    # deal segments (sorted by length desc) snake-wise to cores
    order = sorted(range(len(segs)), key=lambda i: -segs[i][1])
    core_slots = [[] for _ in range(N_CORES)]
    for r, si in enumerate(order):
        c = r % (2 * N_CORES)
        c = c if c < N_CORES else 2 * N_CORES - 1 - c
        core_slots[c].append(si)
    S = max(len(cs) for cs in core_slots)
    # per-slot tile count = max over cores of the slot's segment size
    slot_ts = []
    for k in range(S):
        mx = 1
        for c in range(N_CORES):
            if k < len(core_slots[c]):
                mx = max(mx, (segs[core_slots[c][k]][1] + P - 1) // P)
        slot_ts.append(mx)
    T = sum(slot_ts)
    NC = T * P

    trivial_ln = bool(
        np.all(np.asarray(ln_g) == 1) and np.all(np.asarray(ln_b) == 0)
        and np.all(np.asarray(lin_g) == 1) and np.all(np.asarray(lin_beta) == 0)
    )
    # softmax rows sum to 1, so attn @ (v + 1*bv^T) = attn@v + 1*bv^T; the
    # per-block value bias folds exactly into the output-projection bias:
    # bo_eff = bo + bv @ Wo
    bo = np.asarray(bo, np.float32) + np.einsum(
        "id,idj->ij", np.asarray(bv, np.float32), np.asarray(Wo, np.float32)
    )

    trivial_b = bool(
        np.all(np.asarray(lin_b) == 0) and np.all(np.asarray(bq) == 0)
        and np.all(np.asarray(bk) == 0) and np.all(np.asarray(bo) == 0)
    )
    nc = _get_program(slot_ts, trivial_ln, trivial_b)

    # ---- pack weights ----
    wallp = np.zeros((P, N_WCOLS), np.float32)

    def put_w(base, W):
        W = np.asarray(W, np.float32)
        for k in range(2):
            for j in range(2):
                c0 = base + k * 256 + j * 128
                wallp[:, c0 : c0 + 128] = W[k * 128 : (k + 1) * 128,
                                            j * 128 : (j + 1) * 128]

    put_w(LIN_BASE, lin_W)
    for i in range(2):
        put_w(w_base(i, "q"), np.asarray(Wq)[i])
        put_w(w_base(i, "k"), np.asarray(Wk)[i])
        put_w(w_base(i, "v"), np.asarray(Wv)[i])
        put_w(w_base(i, "o"), np.asarray(Wo)[i])
        # replicated row-mean tiles of Wo for the residual-mean matmuls:
        # mean_d((Wo.T o)_d) = sum_e rowmean(Wo)[e] * o[e]
        wo_rm = np.asarray(Wo)[i].astype(np.float32).mean(axis=1)
        for k in range(2):
            c0 = WO_RM + i * 256 + k * 128
            wallp[:, c0 : c0 + 128] = np.repeat(
                wo_rm[k * 128 : (k + 1) * 128][:, None], 128, axis=1)
    wallp = wallp.astype(np_act)

    consp = np.zeros((P, C_FIXED + T), np.float32)

    def put_c(idx, vec):
        vec = np.asarray(vec, np.float32)
        consp[:, idx] = vec[:128]
        consp[:, idx + 1] = vec[128:]

    put_c(c_lin("b", 0), np.asarray(lin_b, np.float32))
    put_c(c_lin("g", 0), lin_g)
    put_c(c_lin("beta", 0), lin_beta)
    for i in range(2):
        put_c(c_blk(i, "q", 0), np.asarray(bq)[i])
        put_c(c_blk(i, "k", 0), np.asarray(bk)[i])
        put_c(c_blk(i, "v", 0), np.asarray(bv)[i] * 0)
        put_c(c_blk(i, "o", 0), np.asarray(bo)[i])
        put_c(c_blk(i, "g", 0), np.asarray(ln_g)[i])
        put_c(c_blk(i, "beta", 0), np.asarray(ln_b)[i])

    # ---- per-core data ----
    feat_sorted = feat[perm]
    in_maps = []
    core_meta = []  # (slot k, seg_start, seg_len, node_offset in padded layout)
    for c in range(N_CORES):
        fT = np.zeros((NC, 256), np.float32)
        mb = np.full((NC,), NEG, np.float32)
        meta = []
        off = 0
        for k in range(S):
            if k < len(core_slots[c]):
                st, ln = segs[core_slots[c][k]]
                fT[off : off + ln] = feat_sorted[st : st + ln]
                mb[off : off + ln] = 0.0
                meta.append((st, ln, off))
            off += slot_ts[k] * P
        cons_c = consp.copy()
        cons_c[:, C_FIXED : C_FIXED + T] = mb.reshape(T, P).T
        featT_c = np.ascontiguousarray(
            fT.T.reshape(2, P, NC).transpose(1, 0, 2)
        ).astype(np_act)
        in_maps.append({"featT": featT_c, "wall": wallp, "cons": cons_c})
        core_meta.append(meta)

    res = run_bass_kernel_spmd(nc, in_maps, list(range(N_CORES)))
    global _last_results
    _last_results = res

    out = np.empty((N, 256), np.float32)
    for c in range(N_CORES):
        oT = np.asarray(res.results[c]["outT"], np.float32)  # [128, 2, NC]
        o_nodes = oT.transpose(1, 0, 2).reshape(256, NC).T  # [NC, 256]
        for st, ln, off in core_meta[c]:
            out[perm[st : st + ln]] = o_nodes[off : off + ln]
    return out
